# revision 2
# baseline (speedup 1.0000x reference)
"""MoNet v2: table-free Y_k scatter design.

Per layer, per core (dst-sharded, 64-node dst tiles):
 - dma_gather pulls X[src] rows (256B bf16) from DRAM, 4 SWDGE queues
   round-robin so the Q7 pairs emit descriptors concurrently.
 - DVE builds, per 128-edge column, a weighted one-hot block
   ohw (128e, 3*64): ohw[:, k*64+n] = (dl==n) * w_k.
 - PE per column: one matmul Y^T += Xg^T @ ohw accumulated per
   (dst-tile, src-half) into a (IN_C, 192) PSUM: all three Y_k at once.
 - Tile close: ACT copies Y psum -> bf16, then 3 small matmuls
   out += fc_k^T.T @ Y_k fold the kernels; ACT writes (+bias on the lo
   pass), one final DVE add merges the hi pass.
Host: sorts edges by dst, packs columns per (tile, src-half), re-packs
h rows between the two layer launches.
"""

import os
import sys
import numpy as np

QMODE = "rr4"


def _queue_of(b):
    if QMODE == "rr4":
        return b % 4
    if QMODE == "q1":
        return 1
    if QMODE == "rr3":
        return 1 + b % 3
    if QMODE == "rr2":
        return 1 + b % 2 * 2
    raise ValueError(QMODE)

for p in ("/opt/trn_rl_repo",):
    if p not in sys.path:
        sys.path.insert(0, p)

import concourse.bass as bass
import concourse.mybir as mybir
import concourse.tile as tile
from concourse import bacc
from concourse import bass_utils

F32 = mybir.dt.float32
BF16 = mybir.dt.bfloat16
I32 = mybir.dt.int32
I16 = mybir.dt.int16

P = 128
TILE = 64
G = 16


# ----------------------------------------------------------------------------
# Host-side metadata
# ----------------------------------------------------------------------------

def build_edge_metadata(src, dst, pseudo, n_nodes, n_cores):
    NT = -(-n_nodes // P)
    NPAD = NT * P
    NPC = (-(-NT // n_cores)) * P  # nodes per core (last core may be short)
    T = NPC // TILE                # 64-node tiles per core
    HALF = NPAD // 2

    # node -> (core, tile) assignment: snake-deal nodes by degree so every
    # tile's per-half edge count is balanced across cores (shrinks the
    # unified column padding).
    NTILES = n_cores * (NPC // TILE)
    deg = np.bincount(dst, minlength=NPAD)
    nd_order = np.argsort(-deg, kind="stable")
    tile_of_node = np.empty(NPAD, np.int64)
    slot_of_node = np.empty(NPAD, np.int64)
    ar = np.arange(NPAD)
    snake = np.empty(NPAD, np.int64)
    fwd = ar % NTILES
    rev = NTILES - 1 - fwd
    rnd = ar // NTILES
    snake[:] = np.where(rnd % 2 == 0, fwd, rev)
    tile_of_node[nd_order] = snake
    slot_of_node[nd_order] = rnd
    # tiles are dealt globally; map tile -> (core, tloc)
    order = np.argsort(tile_of_node[dst] * TILE + slot_of_node[dst],
                       kind="stable")
    sdst = dst[order]
    ssrc = src[order]
    sps = pseudo[order]

    gt = tile_of_node[sdst]
    core = gt // (NPC // TILE)
    tloc = gt % (NPC // TILE)
    is_hi = (ssrc >= HALF).astype(np.int64)

    # counts per (core, tile, half)
    key = (core * T + tloc) * 2 + is_hi
    cnt = np.bincount(key, minlength=n_cores * T * 2).reshape(n_cores, T, 2)
    C = -(-cnt // P)               # columns per (core, tile, half)

    # per-core column layout: [lo tiles 0..T-1][hi tiles 0..T-1], variable C
    col_off = np.zeros((n_cores, T, 2), np.int64)
    NLO = C[:, :, 0].sum(axis=1)
    TCs = NLO + C[:, :, 1].sum(axis=1)
    for c in range(n_cores):
        col_off[c, :, 0] = np.cumsum(C[c, :, 0]) - C[c, :, 0]
        col_off[c, :, 1] = NLO[c] + np.cumsum(C[c, :, 1]) - C[c, :, 1]
    TC = int(TCs.max())            # same array width for all cores
    NLOmax = NLO  # per-core

    # rank within (core,tile,half)
    starts = np.zeros(n_cores * T * 2, np.int64)
    gcnt = cnt.reshape(-1)
    np.cumsum(gcnt[:-1], out=starts[1:])
    gorder = np.argsort(key, kind="stable")
    rank = np.empty(len(key), np.int64)
    rank[gorder] = np.arange(len(key)) - starts[key[gorder]]

    colg = col_off[core, tloc, is_hi] + rank // P
    pp = rank % P

    src_loc = np.where(is_hi == 0, ssrc, ssrc - HALF).astype(np.int16)
    idx_t = np.zeros((n_cores, P, TC), np.int16)
    dl_t = np.full((n_cores, P, TC), -1.0, np.float32)
    ps_a = np.zeros((n_cores, P, TC), np.float32)
    ps_b = np.zeros((n_cores, P, TC), np.float32)
    idx_t[core, pp, colg] = src_loc
    dl_t[core, pp, colg] = slot_of_node[sdst].astype(np.float32)
    ps_a[core, pp, colg] = sps[:, 0]
    ps_b[core, pp, colg] = sps[:, 1]

    C_u = C.max(axis=0)            # (T, 2) columns per tile, unified
    NLO_v = int(C_u[:, 0].sum()); NHI_v = int(C_u[:, 1].sum())
    TCv = NLO_v + NHI_v
    off_v = np.zeros((T, 2), np.int64)
    off_v[:, 0] = np.cumsum(C_u[:, 0]) - C_u[:, 0]
    off_v[:, 1] = NLO_v + np.cumsum(C_u[:, 1]) - C_u[:, 1]
    idx_v = np.zeros((n_cores, P, TCv), np.int16)
    idx32_v = np.zeros((n_cores, P, TCv), np.int32)
    dl_v = np.full((n_cores, P, TCv), -1.0, np.float32)
    ps_av = np.zeros((n_cores, P, TCv), np.float32)
    ps_bv = np.zeros((n_cores, P, TCv), np.float32)
    colg_v = off_v[tloc, is_hi] + rank // P
    idx_v[core, pp, colg_v] = src_loc
    idx32_v[core, pp, colg_v] = ssrc
    dl_v[core, pp, colg_v] = slot_of_node[sdst].astype(np.float32)
    ps_av[core, pp, colg_v] = sps[:, 0]
    ps_bv[core, pp, colg_v] = sps[:, 1]
    tile_v = np.zeros(TCv, np.int64)
    for t in range(T):
        tile_v[off_v[t, 0]:off_v[t, 0] + C_u[t, 0]] = t
        tile_v[off_v[t, 1]:off_v[t, 1] + C_u[t, 1]] = t

    # batches of G columns within each half
    batches = []   # (c0, gn, is_hi)
    for h, (s0, s1) in enumerate(((0, NLO_v), (NLO_v, TCv))):
        c0 = s0
        while c0 < s1:
            gn = min(G, s1 - c0)
            batches.append((c0, gn, h))
            c0 += gn

    # wrapped int16 idx per batch
    nwrap = sum(gn * P // 16 for _, gn, _ in batches)
    idx_w = np.zeros((n_cores, P, nwrap), np.int16)
    woff = []
    o = 0
    for (c0, gn, h) in batches:
        flat = idx_v[:, :, c0:c0 + gn].transpose(0, 2, 1).reshape(
            n_cores, gn * P)
        w = flat.reshape(n_cores, gn * P // 16, 16).transpose(0, 2, 1)
        idx_w[:, :, o:o + gn * P // 16] = np.tile(w, (1, 8, 1))
        woff.append(o)
        o += gn * P // 16

    return dict(idx_w=idx_w, idx32=idx32_v, dl_t=dl_v, ps_a=ps_av, ps_b=ps_bv,
                batches=batches, woff=woff, tile_v=tile_v,
                C_u=C_u, off_v=off_v, T=T, NT=NT, TCv=TCv, NLO_v=NLO_v,
                nwrap=nwrap,
                _edge_idx=(core, pp, colg_v, slot_of_node[sdst]),
                _sps=sps,
                tile_of_node=tile_of_node, slot_of_node=slot_of_node)


def build_host_ohw(md, pseudo, src, dst, pp_w, pp_b, mu, inv_sigma,
                   n_cores, K=3):
    """Weighted one-hot blocks (n_cores, P, TCv*K*TILE) bf16, built on host."""
    import ml_dtypes
    idx = md["_edge_idx"]          # (core, pp, colg_v, dl) per sorted edge
    core, pp, colg_v, dl = idx
    sps = md["_sps"]
    p0 = np.tanh(sps @ np.asarray(pp_w, np.float64).T
                 + np.asarray(pp_b, np.float64))
    is2 = np.asarray(inv_sigma, np.float64) ** 2
    mu = np.asarray(mu, np.float64)
    a = -0.5 * (is2 * mu ** 2).sum(axis=1)
    w = np.exp(a[None, :]
               + p0 @ (is2 * mu).T
               - 0.5 * (p0 ** 2) @ is2.T)          # (E, K)
    TCv = md["TCv"]
    ohw = np.zeros((n_cores, P, TCv, K, TILE), ml_dtypes.bfloat16)
    for k in range(K):
        ohw[core, pp, colg_v, k, dl] = w[:, k].astype(ml_dtypes.bfloat16)
    return ohw.reshape(n_cores, P, TCv * K * TILE)


def pack_params(pp_w, pp_b, mu, inv_sigma):
    K = mu.shape[0]
    is2 = inv_sigma.astype(np.float64) ** 2
    a = -0.5 * (is2 * mu.astype(np.float64) ** 2).sum(axis=1)
    b = is2 * mu
    c = -0.5 * is2
    par = np.zeros(32, np.float32)
    par[0] = pp_w[0, 0]; par[1] = pp_w[0, 1]
    par[2] = pp_w[1, 0]; par[3] = pp_w[1, 1]
    par[4] = pp_b[0]; par[5] = pp_b[1]
    par[6:6 + K] = a
    par[9:9 + K] = b[:, 0]
    par[12:12 + K] = b[:, 1]
    par[15:15 + K] = c[:, 0]
    par[18:18 + K] = c[:, 1]
    return par.reshape(1, 32)


# ----------------------------------------------------------------------------
# Device kernel builder (one GMMConv layer, transform design)
# ----------------------------------------------------------------------------

HOSTOHW = False
IDMA = False


def build_layer_kernel(md, IN_C, OUT, NPAD, K=3, n_cores=8):
    KOUT = K * OUT
    T = md["T"]
    TCv = md["TCv"]
    HALF = NPAD // 2
    batches = md["batches"]
    woff = md["woff"]
    tile_v = md["tile_v"]
    NWRAP = md["nwrap"]
    ROWE = 128                     # bf16 elems per stored row (256B)

    nc = bacc.Bacc("TRN2", target_bir_lowering=False, debug=False,
                   num_devices=n_cores, num_swdge_queues=4,
                   dynamic_dma_scratch_size=65536)
    rows_d = nc.dram_tensor("rows", [NPAD, ROWE], BF16, kind="ExternalInput")
    fcT_d = nc.dram_tensor("fcT", [IN_C, KOUT], F32, kind="ExternalInput")
    if IDMA:
        idx32_d = nc.dram_tensor("idx32", [P, TCv], I32, kind="ExternalInput")
    else:
        idx_d = nc.dram_tensor("idx_w", [P, NWRAP], I16, kind="ExternalInput")
    if not HOSTOHW:
        dl_d = nc.dram_tensor("dl_t", [P, TCv], F32, kind="ExternalInput")
        psa_d = nc.dram_tensor("ps_a", [P, TCv], F32, kind="ExternalInput")
        psb_d = nc.dram_tensor("ps_b", [P, TCv], F32, kind="ExternalInput")
    par_d = nc.dram_tensor("par", [1, 32], F32, kind="ExternalInput")
    bias_d = nc.dram_tensor("bias", [OUT, 1], F32, kind="ExternalInput")
    if HOSTOHW:
        ohw_d = nc.dram_tensor("ohw_h", [P, TCv * K * TILE], BF16,
                               kind="ExternalInput")
    hout_d = nc.dram_tensor("h_out", [OUT, T * TILE], F32,
                            kind="ExternalOutput")

    with tile.TileContext(nc) as tc:
        with (
            tc.tile_pool(name="const", bufs=1) as cst,
            tc.tile_pool(name="gat", bufs=(10 if G <= 16 else 6)) as gatp,
            tc.tile_pool(name="oh", bufs=3) as ohp,
            tc.tile_pool(name="ohw", bufs=(5 if G <= 16 else 3)) as ohwp,
            tc.tile_pool(name="ysb", bufs=4) as ysbp,
            tc.tile_pool(name="psB", bufs=5, space="PSUM") as psB,
            tc.tile_pool(name="psC", bufs=3, space="PSUM") as psC,
        ):
            # ---- constants ----
            if IDMA:
                idxi_s = cst.tile([P, TCv], I32)
                nc.sync.dma_start(out=idxi_s[:], in_=idx32_d[:])
            else:
                idx_s = cst.tile([P, NWRAP], I16)
                nc.sync.dma_start(out=idx_s[:], in_=idx_d[:])
            spar = cst.tile([P, 32], F32)
            nc.sync.dma_start(out=spar[:], in_=par_d[:].to_broadcast((P, 32)))
            sbias = cst.tile([OUT, 1], F32)
            nc.sync.dma_start(out=sbias[:], in_=bias_d[:])
            zot = cst.tile([OUT, TILE], F32)
            nc.vector.memset(zot[:], 0.0)
            if not HOSTOHW:
                iota_i = cst.tile([P, TILE], I32)
                nc.gpsimd.iota(iota_i[:], pattern=[[1, TILE]], base=0,
                               channel_multiplier=0)
                iota_f = cst.tile([P, TILE], BF16)
                nc.vector.tensor_copy(iota_f[:], iota_i[:])
            fcf = cst.tile([IN_C, KOUT], F32)
            nc.sync.dma_start(out=fcf[:], in_=fcT_d[:])
            fcT = cst.tile([IN_C, KOUT], BF16)
            nc.vector.tensor_copy(fcT[:], fcf[:])
            if not HOSTOHW:
                dl_f = cst.tile([P, TCv], F32)
                nc.sync.dma_start(out=dl_f[:], in_=dl_d[:])
                dl_b = cst.tile([P, TCv], BF16)
                nc.vector.tensor_copy(dl_b[:], dl_f[:])

            # ---- phase W: edge weights wk (P, TCv) bf16 ----
            def ts_mul(out, in0, j):
                nc.vector.tensor_scalar_mul(out, in0, spar[:, j:j + 1])

            wk = [] if HOSTOHW else [
                cst.tile([P, TCv], BF16, name=f"wk{k}", tag=f"w{k}")
                for k in range(K)]
            TC2 = -(-TCv // 2)
            with tc.tile_pool(name="wprep", bufs=1) as wpp:
              if HOSTOHW:
                pass
              else:
               for h0 in range(2):
                cw0 = h0 * TC2
                cwn = min(TC2, TCv - cw0)
                psa_s = wpp.tile([P, TC2], F32, tag="psa", name=f"psa{h0}")
                nc.sync.dma_start(out=psa_s[:, :cwn],
                                  in_=psa_d[:, cw0:cw0 + cwn])
                psb_s = wpp.tile([P, TC2], F32, tag="psb", name=f"psb{h0}")
                nc.sync.dma_start(out=psb_s[:, :cwn],
                                  in_=psb_d[:, cw0:cw0 + cwn])
                pa = wpp.tile([P, TC2], F32, tag="pa", name=f"pa{h0}")
                pb = wpp.tile([P, TC2], F32, tag="pb", name=f"pb{h0}")
                qa = wpp.tile([P, TC2], F32, tag="qa", name=f"qa{h0}")
                qb = wpp.tile([P, TC2], F32, tag="qb", name=f"qb{h0}")
                m1 = wpp.tile([P, TC2], F32, tag="m1", name=f"m1{h0}")
                m2 = wpp.tile([P, TC2], F32, tag="m2", name=f"m2{h0}")
                ts_mul(m1[:, :cwn], psa_s[:, :cwn], 0)
                ts_mul(m2[:, :cwn], psb_s[:, :cwn], 1)
                nc.vector.tensor_add(m1[:, :cwn], m1[:, :cwn], m2[:, :cwn])
                nc.scalar.activation(pa[:, :cwn], m1[:, :cwn],
                                     mybir.ActivationFunctionType.Tanh,
                                     bias=spar[:, 4:5])
                m3 = wpp.tile([P, TC2], F32, tag="m1", name=f"m3{h0}")
                m4 = wpp.tile([P, TC2], F32, tag="m2", name=f"m4{h0}")
                ts_mul(m3[:, :cwn], psa_s[:, :cwn], 2)
                ts_mul(m4[:, :cwn], psb_s[:, :cwn], 3)
                nc.vector.tensor_add(m3[:, :cwn], m3[:, :cwn], m4[:, :cwn])
                nc.scalar.activation(pb[:, :cwn], m3[:, :cwn],
                                     mybir.ActivationFunctionType.Tanh,
                                     bias=spar[:, 5:6])
                nc.scalar.activation(qa[:, :cwn], pa[:, :cwn],
                                     mybir.ActivationFunctionType.Square)
                nc.scalar.activation(qb[:, :cwn], pb[:, :cwn],
                                     mybir.ActivationFunctionType.Square)
                for k in range(K):
                    u1 = wpp.tile([P, TC2], F32, tag="m1", name=f"u1{h0}_{k}")
                    u2 = wpp.tile([P, TC2], F32, tag="m2", name=f"u2{h0}_{k}")
                    u3 = wpp.tile([P, TC2], F32, tag="u3", name=f"u3{h0}_{k}")
                    u4 = wpp.tile([P, TC2], F32, tag="u4", name=f"u4{h0}_{k}")
                    ts_mul(u1[:, :cwn], pa[:, :cwn], 9 + k)
                    ts_mul(u2[:, :cwn], pb[:, :cwn], 12 + k)
                    ts_mul(u3[:, :cwn], qa[:, :cwn], 15 + k)
                    ts_mul(u4[:, :cwn], qb[:, :cwn], 18 + k)
                    nc.vector.tensor_add(u1[:, :cwn], u1[:, :cwn],
                                         u2[:, :cwn])
                    nc.vector.tensor_add(u3[:, :cwn], u3[:, :cwn],
                                         u4[:, :cwn])
                    nc.vector.tensor_add(u1[:, :cwn], u1[:, :cwn],
                                         u3[:, :cwn])
                    nc.scalar.activation(wk[k][:, cw0:cw0 + cwn],
                                         u1[:, :cwn],
                                         mybir.ActivationFunctionType.Exp,
                                         bias=spar[:, 6 + k:7 + k])

            # ---- phase B ----
            h_sbuf = cst.tile([OUT, T * TILE], F32)

            NB = len(batches)
            state = {}   # (half, tile) -> Y psum tile (IN_C, K*TILE)

            def do_gather(b):
                c0, gn, h = batches[b]
                gat = gatp.tile([P, G * ROWE], BF16, tag="gat",
                                name=f"gat{b}")
                if IDMA:
                    nc.gpsimd.indirect_dma_start(
                        out=gat[:].rearrange("p (j f) -> p j f",
                                             f=ROWE)[:, :gn, :],
                        out_offset=None,
                        in_=rows_d[:, :],
                        in_offset=bass.IndirectOffsetOnAxis(
                            ap=idxi_s[:, c0:c0 + gn], axis=0),
                    )
                    return gat
                src_half = rows_d[0:HALF, :] if h == 0 else \
                    rows_d[HALF:NPAD, :]
                nc.gpsimd.dma_gather(
                    out_ap=gat[:].rearrange("p (j f) -> p j f",
                                            f=ROWE)[:, :gn, :],
                    in_ap=src_half,
                    idxs_ap=idx_s[:, woff[b]:woff[b] + gn * P // 16],
                    num_idxs=gn * P, num_idxs_reg=gn * P,
                    elem_size=ROWE, single_packet=False, queue_num=_queue_of(b))
                return gat

            def do_ohw(b):
                c0, gn, h = batches[b]
                if HOSTOHW:
                    ohw = ohwp.tile([P, G * K * TILE], BF16, tag="ohw",
                                    name=f"ohw{b}")
                    nc.sync.dma_start(
                        out=ohw[:, 0:gn * K * TILE],
                        in_=ohw_d[:, c0 * K * TILE:(c0 + gn) * K * TILE])
                    return ohw
                oh = ohp.tile([P, G * TILE], BF16, tag="oh", name=f"oh{b}")
                nc.vector.tensor_tensor(
                    out=oh[:].rearrange("p (j n) -> p j n",
                                        n=TILE)[:, :gn, :],
                    in0=dl_b[:, c0:c0 + gn].to_broadcast((P, gn, TILE)),
                    in1=iota_f[:].rearrange("p (j n) -> p j n",
                                            j=1).to_broadcast((P, gn, TILE)),
                    op=mybir.AluOpType.is_equal)
                ohw = ohwp.tile([P, G * K * TILE], BF16, tag="ohw",
                                name=f"ohw{b}")
                ohw3 = ohw[:].rearrange("p (j q) -> p j q", q=K * TILE)
                for k in range(K):
                    nc.vector.tensor_tensor(
                        out=ohw3[:, :gn, k * TILE:(k + 1) * TILE],
                        in0=oh[:].rearrange("p (j n) -> p j n",
                                            n=TILE)[:, :gn, :],
                        in1=wk[k][:, c0:c0 + gn].to_broadcast((P, gn, TILE)),
                        op=mybir.AluOpType.mult)
                return ohw

            def close_tile(key):
                h, t = key
                ysb = ysbp.tile([IN_C, K * TILE], BF16, tag="ysb",
                                name=f"ysb{h}_{t}")
                nc.scalar.activation(ysb[:], state[key][:],
                                     mybir.ActivationFunctionType.Copy)
                ops = psC.tile([OUT, TILE], F32, tag="ops",
                               name=f"ops{h}_{t}")
                for k in range(K):
                    nc.tensor.matmul(
                        ops[:],
                        lhsT=fcT[:, k * OUT:(k + 1) * OUT],
                        rhs=ysb[:, k * TILE:(k + 1) * TILE],
                        start=(k == 0), stop=(k == K - 1))
                if h == 0:
                    nc.scalar.activation(
                        h_sbuf[:, t * TILE:(t + 1) * TILE], ops[:],
                        mybir.ActivationFunctionType.Identity,
                        bias=sbias[:, 0:1])
                else:
                    nc.vector.tensor_add(
                        h_sbuf[:, t * TILE:(t + 1) * TILE],
                        h_sbuf[:, t * TILE:(t + 1) * TILE], ops[:])
                del state[key]

            def do_scatter(b, gat, ohw):
                c0, gn, h = batches[b]
                ohw3 = ohw[:].rearrange("p (j q) -> p j q", q=K * TILE)
                for j in range(gn):
                    c = c0 + j
                    t = int(tile_v[c])
                    key = (h, t)
                    first = key not in state
                    if first:
                        state[key] = psB.tile([IN_C, K * TILE], F32,
                                              tag="acc", name=f"acc{h}_{t}")
                    last = (c == last_col[key])
                    nc.tensor.matmul(
                        state[key][:],
                        lhsT=gat[:, j * ROWE:j * ROWE + IN_C],
                        rhs=ohw3[:, j, :],
                        start=first, stop=last)
                    if last:
                        close_tile(key)

            last_col = {}
            C_u, off_v = md["C_u"], md["off_v"]
            for t in range(T):
                if C_u[t, 0] > 0:
                    last_col[(0, t)] = int(off_v[t, 0] + C_u[t, 0] - 1)
                if C_u[t, 1] > 0:
                    last_col[(1, t)] = int(off_v[t, 1] + C_u[t, 1] - 1)

            # software pipeline: gathers LOOKG ahead, ohw one ahead
            gat_q = {}
            ohw_q = {}
            LOOKG, LOOKO = (8, 2) if G <= 16 else (5, 2)
            for b in range(min(LOOKG, NB)):
                gat_q[b] = do_gather(b)
            for b in range(min(LOOKO, NB)):
                ohw_q[b] = do_ohw(b)
            for b in range(NB):
                nb_g = b + LOOKG
                if nb_g < NB:
                    gat_q[nb_g] = do_gather(nb_g)
                nb_o = b + LOOKO
                if nb_o < NB:
                    ohw_q[nb_o] = do_ohw(nb_o)
                do_scatter(b, gat_q[b], ohw_q[b])
                gat_q.pop(b, None)
                ohw_q.pop(b, None)

            # zero-edge tiles: emit bias-only output
            for t in range(T):
                if C_u[t, 0] == 0:
                    nc.vector.tensor_scalar(
                        out=h_sbuf[:, t * TILE:(t + 1) * TILE],
                        in0=zot[:], scalar1=sbias[:, 0:1], scalar2=None,
                        op0=mybir.AluOpType.add)
            nc.sync.dma_start(out=hout_d[:], in_=h_sbuf[:])

    nc.compile()
    return nc


# ----------------------------------------------------------------------------
# Full model runner
# ----------------------------------------------------------------------------

_KERNEL_CACHE = {}


def _get_kernel(key, builder):
    if key not in _KERNEL_CACHE:
        _KERNEL_CACHE[key] = builder()
    return _KERNEL_CACHE[key]


def _rows_bf16(x, NPAD, ROWE=128):
    import ml_dtypes
    n, f = x.shape
    out = np.zeros((NPAD, ROWE), ml_dtypes.bfloat16)
    out[:n, :f] = x.astype(ml_dtypes.bfloat16)
    return out


def run_monet(inputs, n_cores=8, trace=False):
    feat = np.asarray(inputs["feat"], np.float32)
    pseudo = np.asarray(inputs["pseudo"], np.float32)
    src = np.asarray(inputs["src"], np.int32)
    dst = np.asarray(inputs["dst"], np.int32)
    N, IN_F = feat.shape
    HID = np.asarray(inputs["fc0"]).shape[0] // 3
    OUTF = np.asarray(inputs["fc1"]).shape[0] // 3
    K = 3

    md = build_edge_metadata(src, dst, pseudo, N, n_cores)
    NT = md["NT"]
    NPAD = NT * P
    T = md["T"]

    rows0 = _rows_bf16(feat, NPAD)
    fc0T = np.ascontiguousarray(np.asarray(inputs["fc0"], np.float32).T)
    fc1T = np.ascontiguousarray(np.asarray(inputs["fc1"], np.float32).T)
    par0 = pack_params(np.asarray(inputs["pp0_w"], np.float32),
                       np.asarray(inputs["pp0_b"], np.float32),
                       np.asarray(inputs["mu0"], np.float32),
                       np.asarray(inputs["inv_sigma0"], np.float32))
    par1 = pack_params(np.asarray(inputs["pp1_w"], np.float32),
                       np.asarray(inputs["pp1_b"], np.float32),
                       np.asarray(inputs["mu1"], np.float32),
                       np.asarray(inputs["inv_sigma1"], np.float32))
    b0 = np.asarray(inputs["b0"], np.float32).reshape(HID, 1)
    b1 = np.asarray(inputs["b1"], np.float32).reshape(OUTF, 1)

    hostohw = False
    if hostohw:
        ohw0 = build_host_ohw(md, pseudo, src, dst,
                              np.asarray(inputs["pp0_w"], np.float32),
                              np.asarray(inputs["pp0_b"], np.float32),
                              np.asarray(inputs["mu0"], np.float32),
                              np.asarray(inputs["inv_sigma0"], np.float32),
                              n_cores, K)
        ohw1 = build_host_ohw(md, pseudo, src, dst,
                              np.asarray(inputs["pp1_w"], np.float32),
                              np.asarray(inputs["pp1_b"], np.float32),
                              np.asarray(inputs["mu1"], np.float32),
                              np.asarray(inputs["inv_sigma1"], np.float32),
                              n_cores, K)

    kkey0 = ("v2l0", NT, md["TCv"], IN_F, HID, hostohw, "0")
    nc0 = _get_kernel(kkey0, lambda: build_layer_kernel(md, IN_F, HID, NPAD,
                                                        K, n_cores))
    in_maps0 = []
    for c in range(n_cores):
        idma = False
        m = dict(rows=rows0, fcT=fc0T, par=par0, bias=b0)
        if idma:
            m["idx32"] = md["idx32"][c]
        else:
            m["idx_w"] = md["idx_w"][c]
        if hostohw:
            m["ohw_h"] = ohw0[c]
        else:
            m.update(dl_t=md["dl_t"][c], ps_a=md["ps_a"][c],
                     ps_b=md["ps_b"][c])
        in_maps0.append(m)
    res0 = bass_utils.run_bass_kernel_spmd(
        nc0, in_maps0, core_ids=list(range(n_cores)), trace=trace)

    hcat = np.concatenate([res0.results[c]["h_out"] for c in range(n_cores)],
                          axis=1)
    gpos = md["tile_of_node"] * TILE + md["slot_of_node"]   # (NPAD,)
    h_nodes = hcat[:, gpos[:N]]                             # (HID, N)
    rows1 = _rows_bf16(np.ascontiguousarray(h_nodes.T), NPAD)

    kkey1 = ("v2l1", NT, md["TCv"], HID, OUTF, hostohw, "0")
    nc1 = _get_kernel(kkey1, lambda: build_layer_kernel(md, HID, OUTF, NPAD,
                                                        K, n_cores))
    in_maps1 = []
    for c in range(n_cores):
        idma = False
        m = dict(rows=rows1, fcT=fc1T, par=par1, bias=b1)
        if idma:
            m["idx32"] = md["idx32"][c]
        else:
            m["idx_w"] = md["idx_w"][c]
        if hostohw:
            m["ohw_h"] = ohw1[c]
        else:
            m.update(dl_t=md["dl_t"][c], ps_a=md["ps_a"][c],
                     ps_b=md["ps_b"][c])
        in_maps1.append(m)
    res1 = bass_utils.run_bass_kernel_spmd(
        nc1, in_maps1, core_ids=list(range(n_cores)), trace=trace)

    ocat = np.concatenate([res1.results[c]["h_out"] for c in range(n_cores)],
                          axis=1)
    out = np.ascontiguousarray(ocat[:, gpos[:N]].T)
    perf = dict(l0_ns=res0.exec_time_ns, l1_ns=res1.exec_time_ns)
    return out, perf


# ----------------------------------------------------------------------------
# Harness entry: full inputs in, full output out
# ----------------------------------------------------------------------------

def kernel(**inputs):
    out, _ = run_monet(inputs)
    return out.astype(np.float32)
